# revision 8
# baseline (speedup 1.0000x reference)
"""Causal multi-head self-attention on 8 TRN2 NeuronCores.

Sharding: tensor-parallel over heads. 16 heads / 8 cores = 2 heads per core.
Each core computes q/k/v projections for its 2 heads (fp16 matmuls),
block-causal attention (scores kept k-major so softmax sums come from a
fused ones-column in the attn@v matmul), and a partial output projection
against its 128-column slice of W_O. The host sums the 8 fp32 partials.

All matmul operands are fp16 (1.0 cycles/row on the PE, same as wide fp32r,
but half the DMA bytes and LDWEIGHTS time). PSUM accumulation stays fp32.
Scores use K=64 matmuls with partition-base offsets per head (measured on
HW: same per-column rate as K=128, so no zero-padding needed). exp runs on
ScalarE only (scale=1/sqrt(dk), bias=-1 range centering), writing fp16.

vtok layout per k-tile: [vA(0:64) | onesA@64 | .. | onesB@128 | .. |
vB(192:256)], so head A ctx lands on PSUM partitions 0:64 (denom row 64)
and head B ctx on partitions 64:128 (denom row 0) — the ctx evacuation is
two same-partition copies, no cross-partition DMA. The two denominator
rows are fp32r-cast to SBUF, broadcast to [128, QTILE] with two K=1
matmuls (ind rows live on partitions 64 / 0), reciprocal'd with the fast
custom-DVE op, and the normalize multiply runs on the idle Pool engine.

The output projection DMAs PSUM directly to DRAM (fp32) — no engine copy.

Schedule: stage A (projections) of batch b+1 is interleaved per-qtile into
stage B (attention) of batch b, so the PE fills ScalarE-bound exp gaps
with projection matmuls and batch transitions have no PE bubble. Deferred
rb/normalize/oproj of attention tile qi run inside the next interleaved
stage-A qtile.
"""

import numpy as np
from contextlib import ExitStack

import concourse.bass as bass
import concourse.tile as tile
from concourse import bacc, mybir
from concourse.bass_utils import run_bass_kernel_spmd

F32 = mybir.dt.float32
F32R = mybir.dt.float32r
F16 = mybir.dt.float16

B, S, D, H = 4, 2048, 1024, 16
DK = D // H  # 64
NCORES = 8
T = B * S  # 8192 tokens
KT = D // 128  # 8 contraction tiles for projections
QTILE = 512  # q-tile width (tokens)
KTILE = 128  # k-tile width (tokens)
NQT = S // QTILE  # 4 q-tiles per batch
NKT = S // KTILE  # 16 k-tiles per batch
LAG = 2  # ctx matmuls trail scores by this many k-tiles
EXP_FUNC = mybir.ActivationFunctionType.Exp
INV_SQRT_DK = 1.0 / np.sqrt(DK)


def build_nc():
    nc = bacc.Bacc("TRN2", target_bir_lowering=False, debug=False)

    xT = nc.dram_tensor("xT", [D, T], F16, kind="ExternalInput").ap()
    wq = nc.dram_tensor("wq", [D, 128], F16, kind="ExternalInput").ap()
    wk = nc.dram_tensor("wk", [D, 128], F16, kind="ExternalInput").ap()
    wv = nc.dram_tensor("wv", [D, 128], F16, kind="ExternalInput").ap()
    wo = nc.dram_tensor("wo", [128, D], F16, kind="ExternalInput").ap()
    tri = nc.dram_tensor("tri", [128, 128], F16, kind="ExternalInput").ap()
    ind = nc.dram_tensor("ind", [65, 128], F16, kind="ExternalInput").ap()
    ident = nc.dram_tensor("ident", [128, 128], F16, kind="ExternalInput").ap()
    outT = nc.dram_tensor("outT", [D, T], F16, kind="ExternalOutput").ap()

    with ExitStack() as ctx:
        tc = ctx.enter_context(tile.TileContext(nc))
        consts = ctx.enter_context(tc.tile_pool(name="consts", bufs=1))
        xt_pool = ctx.enter_context(tc.tile_pool(name="xt_pool", bufs=2))
        batch_pool = ctx.enter_context(tc.tile_pool(name="batch_pool", bufs=2))
        vtmp_pool = ctx.enter_context(tc.tile_pool(name="vtmp_pool", bufs=3))
        exp_pool = ctx.enter_context(tc.tile_pool(name="exp_pool", bufs=4))
        ctxn_pool = ctx.enter_context(tc.tile_pool(name="ctxn_pool", bufs=2))
        oall_pool = ctx.enter_context(tc.tile_pool(name="oall_pool", bufs=2))
        small_pool = ctx.enter_context(tc.tile_pool(name="small_pool", bufs=2))
        ps = ctx.enter_context(tc.tile_pool(name="ps", bufs=1, space="PSUM"))

        # --- constants / weights (persistent) ---
        wq_sb = consts.tile([128, KT, 128], F16)
        nc.sync.dma_start(out=wq_sb, in_=wq.rearrange("(kt p) m -> p kt m", p=128))
        wk_sb = consts.tile([128, KT, 128], F16)
        nc.sync.dma_start(out=wk_sb, in_=wk.rearrange("(kt p) m -> p kt m", p=128))
        wv_sb = consts.tile([128, KT, 128], F16)
        nc.sync.dma_start(out=wv_sb, in_=wv.rearrange("(kt p) m -> p kt m", p=128))
        wo_sb = consts.tile([128, KT, 128], F16)
        nc.sync.dma_start(out=wo_sb, in_=wo.rearrange("p (jt m) -> p jt m", jt=KT))
        tri_sb = consts.tile([128, 128], F16)
        nc.sync.dma_start(out=tri_sb, in_=tri)
        # ind row 64: ones on cols 0:64 (head A denom -> partitions 0:64)
        # ind row 0: ones on cols 64:128 (head B denom -> partitions 64:128)
        ind_sb = consts.tile([65, 128], F16)
        nc.sync.dma_start(out=ind_sb, in_=ind)
        ident_sb = consts.tile([128, 128], F16)
        nc.sync.dma_start(out=ident_sb, in_=ident)
        biasm1 = consts.tile([128, 1], F32)
        nc.vector.memset(biasm1, -1.0)

        pend_rb = []
        pend_oproj = []

        def pop_rb():
            if pend_rb:
                pend_rb.pop(0)()

        def pop_oproj():
            if pend_oproj:
                pend_oproj.pop(0)()

        batch_tiles = {}

        def alloc_batch(b):
            qT_sb = batch_pool.tile([128, S], F16, name="qT_sb")
            kT_sb = batch_pool.tile([128, S], F16, name="kT_sb")
            vtok_sb = batch_pool.tile([128, NKT, 256], F16, name="vtok_sb")
            nc.gpsimd.memset(vtok_sb[:, :, 64:65], 1.0)
            nc.gpsimd.memset(vtok_sb[:, :, 128:129], 1.0)
            batch_tiles[b] = (qT_sb, kT_sb, vtok_sb)

        def stageA_chunks(b, tt):
            """Projections + v transpose for qtile tt of batch b, as a list
            of small closures to interleave into the attention loop."""
            chunks = []

            def proj(w_sb, done):
                pt = {}

                def mm0():
                    qT_sb, kT_sb, vtok_sb = batch_tiles[b]
                    t0 = b * S + tt * QTILE
                    xt = stA.get("xt")
                    if xt is None:
                        xt = xt_pool.tile([128, KT, QTILE], F16, name="xt", tag="xt")
                        nc.sync.dma_start(
                            out=xt,
                            in_=xT.rearrange("(kt p) t -> p kt t", p=128)[
                                :, :, t0 : t0 + QTILE
                            ],
                        )
                        stA["xt"] = xt
                    pt["p"] = ps.tile([128, QTILE], F32, name="pP", tag="mm", bufs=2)
                    for kt in range(KT // 2):
                        nc.tensor.matmul(
                            pt["p"], w_sb[:, kt, :], xt[:, kt, :],
                            start=(kt == 0), stop=False, skip_group_check=True,
                        )

                def mm1():
                    xt = stA["xt"]
                    for kt in range(KT // 2, KT):
                        nc.tensor.matmul(
                            pt["p"], w_sb[:, kt, :], xt[:, kt, :],
                            start=False, stop=(kt == KT - 1), skip_group_check=True,
                        )
                    done(pt["p"])

                chunks.append(mm0)
                chunks.append(mm1)

            stA = {}

            def q_done(p):
                qT_sb = batch_tiles[b][0]
                nc.vector.tensor_copy(qT_sb[:, tt * QTILE : (tt + 1) * QTILE], p)

            def k_done(p):
                kT_sb = batch_tiles[b][1]
                nc.vector.tensor_copy(kT_sb[:, tt * QTILE : (tt + 1) * QTILE], p)

            def v_done(p):
                vT_tmp = vtmp_pool.tile([128, QTILE], F16, name="vT_tmp")
                nc.vector.tensor_copy(vT_tmp, p)
                stA["vT"] = vT_tmp

            proj(wq_sb, q_done)
            chunks.append(pop_rb)
            proj(wk_sb, k_done)
            chunks.append(pop_oproj)
            proj(wv_sb, v_done)

            def vtr(sp):
                def run():
                    vtok_sb = batch_tiles[b][2]
                    vT_tmp = stA["vT"]
                    vtokP = ps.tile([128, 2, 128], F16, name="vtokP", tag="mm", bufs=2)
                    for j in range(2):
                        s = sp * 2 + j
                        nc.tensor.transpose(
                            vtokP[:, j, :], vT_tmp[:, s * 128 : (s + 1) * 128], ident_sb
                        )
                    m0 = tt * 4 + sp * 2
                    nc.vector.tensor_copy(
                        vtok_sb[:, m0 : m0 + 2, 0:64], vtokP[:, :, 0:64]
                    )
                    nc.vector.tensor_copy(
                        vtok_sb[:, m0 : m0 + 2, 192:256], vtokP[:, :, 64:128]
                    )

                return run

            for sp in range(QTILE // 256):
                chunks.append(vtr(sp))
            return chunks

        def attention_qtile(b, qi, feed=()):
            """Block-causal attention for q-tile qi of batch b; pops one
            feed chunk per k-iteration to keep the PE saturated. Appends the
            deferred rb/oproj closures."""
            feed = list(feed)
            qT_sb, kT_sb, vtok_sb = batch_tiles[b]
            tb = b * S
            q0 = qi * QTILE  # batch-local q base
            nk = 4 * qi + 4  # k-tiles for this q-tile (block-causal)
            ctxAB = ps.tile([128, 2 * QTILE], F32, name="ctxAB", tag="ctx", bufs=1)

            def geom(m, qi=qi):
                d_off = m - 4 * qi
                if d_off >= 0:
                    return QTILE - 128 * d_off, 128 * d_off, True
                return QTILE, 0, False

            exps = {}
            for i in range(nk + LAG):
                if i >= 1 and feed:
                    feed.pop(0)()
                if i < nk:
                    m = i
                    width, qoff, diag = geom(m)
                    sAB = ps.tile([128, 2 * QTILE], F32, name="sAB", tag="sc", bufs=2)
                    nc.tensor.matmul(
                        sAB[:, 0:width],
                        kT_sb[0:64, m * 128 : (m + 1) * 128],
                        qT_sb[0:64, q0 + qoff : q0 + QTILE],
                        start=True,
                        stop=True,
                    )
                    nc.tensor.matmul(
                        sAB[:, QTILE : QTILE + width],
                        kT_sb[64:128, m * 128 : (m + 1) * 128],
                        qT_sb[64:128, q0 + qoff : q0 + QTILE],
                        start=True,
                        stop=True,
                    )
                    # one wide exp covers both heads; the gap region
                    # [width:QTILE] holds unread junk for diag tiles.
                    eAB = exp_pool.tile([128, 2 * QTILE], F16, name="eAB", tag="exp")
                    nc.scalar.activation(
                        eAB[:, 0 : QTILE + width],
                        sAB[:, 0 : QTILE + width],
                        EXP_FUNC,
                        scale=INV_SQRT_DK,
                        bias=biasm1,
                    )
                    if diag:
                        nc.gpsimd.tensor_mul(eAB[:, 0:128], eAB[:, 0:128], tri_sb)
                        nc.gpsimd.tensor_mul(
                            eAB[:, QTILE : QTILE + 128],
                            eAB[:, QTILE : QTILE + 128],
                            tri_sb,
                        )
                    exps[m] = eAB

                j = i - LAG
                if j >= 0:
                    width, qoff, _ = geom(j)
                    first = j == 0
                    last = j == nk - 1
                    eAB = exps.pop(j)
                    nc.tensor.matmul(
                        ctxAB[:, qoff:QTILE],
                        vtok_sb[:, j, 0:128],
                        eAB[:, 0:width],
                        start=first,
                        stop=last,
                        skip_group_check=True,
                    )
                    nc.tensor.matmul(
                        ctxAB[:, QTILE + qoff : 2 * QTILE],
                        vtok_sb[:, j, 128:256],
                        eAB[:, QTILE : QTILE + width],
                        start=first,
                        stop=last,
                        skip_group_check=True,
                    )

            while feed:
                feed.pop(0)()

            # evacuate the two denominator rows: head A at (64, first half),
            # head B at (0, second half). Same-partition copies, fp16-cast.
            s2 = small_pool.tile([65, 2, QTILE], F16, name="s2")
            nc.vector.tensor_copy(s2[64:65, 0, :], ctxAB[64:65, 0:QTILE])
            nc.vector.tensor_copy(s2[0:1, 1, :], ctxAB[0:1, QTILE : 2 * QTILE])
            ctxn = ctxn_pool.tile([128, QTILE], F16, name="ctxn")

            def finish_rb(ctxn=ctxn, s2=s2, ctxAB=ctxAB):
                # broadcast raw denominators via two K=1 fp32r matmuls (ind
                # rows sit on partitions 64 / 0), one fast reciprocal, then
                # normalize fused into the ctx PSUM evacuation copies.
                rbP = ps.tile([128, QTILE], F32, name="rbP", tag="mm", bufs=2)
                nc.tensor.matmul(
                    rbP[0:64, :], ind_sb[64:65, 0:64], s2[64:65, 0, :],
                    start=True, stop=True,
                )
                nc.tensor.matmul(
                    rbP[64:128, :], ind_sb[0:1, 64:128], s2[0:1, 1, :],
                    start=True, stop=True, skip_group_check=True,
                )
                rb_sb = small_pool.tile([128, QTILE], F32, name="rb_sb")
                nc.vector.reciprocal_approx_fast(rb_sb, rbP)
                with nc.allow_low_precision(reason="softmax normalize"):
                    nc.vector.tensor_mul(
                        ctxn[0:64, :], ctxAB[0:64, 0:QTILE], rb_sb[0:64, :]
                    )
                    nc.vector.tensor_mul(
                        ctxn[64:128, :],
                        ctxAB[64:128, QTILE : 2 * QTILE],
                        rb_sb[64:128, :],
                    )

            def finish_oproj(q0=q0, tb=tb, ctxn=ctxn):
                o_all = oall_pool.tile([128, KT, QTILE], F16, name="o_all")
                for jt in range(KT):
                    oP = ps.tile([128, QTILE], F32, name="oP", tag="mm", bufs=2)
                    nc.tensor.matmul(oP, wo_sb[:, jt, :], ctxn, start=True, stop=True)
                    nc.any.tensor_copy(o_all[:, jt, :], oP)
                nc.sync.dma_start(
                    out=outT.rearrange("(jt p) t -> p jt t", p=128)[
                        :, :, tb + q0 : tb + q0 + QTILE
                    ],
                    in_=o_all,
                )

            pend_rb.append(finish_rb)
            pend_oproj.append(finish_oproj)

        # --- schedule: stage A of batch b+1 is interleaved chunk-by-chunk
        # into the attention k-loops of batch b.
        alloc_batch(0)
        for tt in range(NQT):
            for c in stageA_chunks(0, tt):
                c()
        for b in range(B):
            if b + 1 < B:
                alloc_batch(b + 1)
            for qi in range(NQT):
                if b + 1 < B:
                    feed = stageA_chunks(b + 1, qi)
                else:
                    feed = [pop_rb, pop_oproj]
                attention_qtile(b, qi, feed=feed)

        while pend_rb:
            pend_rb.pop(0)()
        while pend_oproj:
            pend_oproj.pop(0)()

    nc.compile()
    return nc


_NC = None


def _get_nc():
    global _NC
    if _NC is None:
        _NC = build_nc()
    return _NC


def make_in_maps(x, W_Q, W_K, W_V, W_O):
    xTh = np.ascontiguousarray(
        np.asarray(x, dtype=np.float32).reshape(T, D).T
    ).astype(np.float16)
    W_Q = np.asarray(W_Q, dtype=np.float32)
    W_K = np.asarray(W_K, dtype=np.float32)
    W_V = np.asarray(W_V, dtype=np.float32)
    W_O = np.asarray(W_O, dtype=np.float32)
    tri = np.triu(np.ones((128, 128), dtype=np.float16))  # tri[k,q]=1 iff q>=k
    ind2 = np.zeros((65, 128), dtype=np.float16)
    ind2[64, 0:64] = 1.0  # head A denom row (partition 64) -> out parts 0:64
    ind2[0, 64:128] = 1.0  # head B denom row (partition 0) -> out parts 64:128
    ident = np.eye(128, dtype=np.float16)
    in_maps = []
    for c in range(NCORES):
        sl = slice(c * 128, (c + 1) * 128)
        in_maps.append(
            {
                "xT": xTh,
                "wq": np.ascontiguousarray(W_Q[sl, :].T).astype(np.float16),
                "wk": np.ascontiguousarray(W_K[sl, :].T).astype(np.float16),
                "wv": np.ascontiguousarray(W_V[sl, :].T).astype(np.float16),
                "wo": np.ascontiguousarray(W_O.T[sl, :]).astype(np.float16),
                "tri": tri,
                "ind": ind2,
                "ident": ident,
            }
        )
    return in_maps


def kernel(x, W_Q, W_K, W_V, W_O, _results_hook=None):
    nc = _get_nc()
    in_maps = make_in_maps(x, W_Q, W_K, W_V, W_O)
    res = run_bass_kernel_spmd(nc, in_maps, list(range(NCORES)))
    if _results_hook is not None:
        _results_hook(res)
    acc = np.zeros((D, T), dtype=np.float32)
    for c in range(NCORES):
        acc += res.results[c]["outT"].astype(np.float32)
    out = np.ascontiguousarray(acc.T).reshape(B, S, D)
    return out


# revision 9
# speedup vs baseline: 1.0079x; 1.0079x over previous
"""Causal multi-head self-attention on 8 TRN2 NeuronCores.

Sharding: tensor-parallel over heads. 16 heads / 8 cores = 2 heads per core.
Each core computes q/k/v projections for its 2 heads (fp16 matmuls),
block-causal attention (scores kept k-major so softmax sums come from a
fused ones-column in the attn@v matmul), and a partial output projection
against its 128-column slice of W_O. The host sums the 8 fp32 partials.

All matmul operands are fp16 (1.0 cycles/row on the PE, same as wide fp32r,
but half the DMA bytes and LDWEIGHTS time). PSUM accumulation stays fp32.
Scores use K=64 matmuls with partition-base offsets per head (measured on
HW: same per-column rate as K=128, so no zero-padding needed). exp runs on
ScalarE only (scale=1/sqrt(dk), bias=-1 range centering), writing fp16.

vtok layout per k-tile: [vA(0:64) | onesA@64 | .. | onesB@128 | .. |
vB(192:256)], so head A ctx lands on PSUM partitions 0:64 (denom row 64)
and head B ctx on partitions 64:128 (denom row 0) — the ctx evacuation is
two same-partition copies, no cross-partition DMA. The two denominator
rows are fp32r-cast to SBUF, broadcast to [128, QTILE] with two K=1
matmuls (ind rows live on partitions 64 / 0), reciprocal'd with the fast
custom-DVE op, and the normalize multiply runs on the idle Pool engine.

The output projection DMAs PSUM directly to DRAM (fp32) — no engine copy.

Schedule: stage A (projections) of batch b+1 is interleaved per-qtile into
stage B (attention) of batch b, so the PE fills ScalarE-bound exp gaps
with projection matmuls and batch transitions have no PE bubble. Deferred
rb/normalize/oproj of attention tile qi run inside the next interleaved
stage-A qtile.
"""

import numpy as np
from contextlib import ExitStack

import concourse.bass as bass
import concourse.tile as tile
from concourse import bacc, mybir
from concourse.bass_utils import run_bass_kernel_spmd

F32 = mybir.dt.float32
F32R = mybir.dt.float32r
F16 = mybir.dt.float16

B, S, D, H = 4, 2048, 1024, 16
DK = D // H  # 64
NCORES = 8
T = B * S  # 8192 tokens
KT = D // 128  # 8 contraction tiles for projections
QTILE = 512  # q-tile width (tokens)
KTILE = 128  # k-tile width (tokens)
NQT = S // QTILE  # 4 q-tiles per batch
NKT = S // KTILE  # 16 k-tiles per batch
LAG = 2  # ctx matmuls trail scores by this many k-tiles
EXP_FUNC = mybir.ActivationFunctionType.Exp
INV_SQRT_DK = 1.0 / np.sqrt(DK)


def build_nc():
    nc = bacc.Bacc("TRN2", target_bir_lowering=False, debug=False)

    xT = nc.dram_tensor("xT", [D, T], F16, kind="ExternalInput").ap()
    wq = nc.dram_tensor("wq", [D, 128], F16, kind="ExternalInput").ap()
    wk = nc.dram_tensor("wk", [D, 128], F16, kind="ExternalInput").ap()
    wv = nc.dram_tensor("wv", [D, 128], F16, kind="ExternalInput").ap()
    wo = nc.dram_tensor("wo", [128, D], F16, kind="ExternalInput").ap()
    tri = nc.dram_tensor("tri", [128, 128], F16, kind="ExternalInput").ap()
    ind = nc.dram_tensor("ind", [65, 128], F16, kind="ExternalInput").ap()
    ident = nc.dram_tensor("ident", [128, 128], F16, kind="ExternalInput").ap()
    outT = nc.dram_tensor("outT", [D, T], F16, kind="ExternalOutput").ap()

    with ExitStack() as ctx:
        tc = ctx.enter_context(tile.TileContext(nc))
        consts = ctx.enter_context(tc.tile_pool(name="consts", bufs=1))
        xt_pool = ctx.enter_context(tc.tile_pool(name="xt_pool", bufs=2))
        batch_pool = ctx.enter_context(tc.tile_pool(name="batch_pool", bufs=2))
        vtmp_pool = ctx.enter_context(tc.tile_pool(name="vtmp_pool", bufs=3))
        exp_pool = ctx.enter_context(tc.tile_pool(name="exp_pool", bufs=4))
        ctxn_pool = ctx.enter_context(tc.tile_pool(name="ctxn_pool", bufs=2))
        oall_pool = ctx.enter_context(tc.tile_pool(name="oall_pool", bufs=2))
        small_pool = ctx.enter_context(tc.tile_pool(name="small_pool", bufs=2))
        ps = ctx.enter_context(tc.tile_pool(name="ps", bufs=1, space="PSUM"))

        # --- constants / weights (persistent) ---
        wq_sb = consts.tile([128, KT, 128], F16)
        nc.sync.dma_start(out=wq_sb, in_=wq.rearrange("(kt p) m -> p kt m", p=128))
        wk_sb = consts.tile([128, KT, 128], F16)
        nc.sync.dma_start(out=wk_sb, in_=wk.rearrange("(kt p) m -> p kt m", p=128))
        wv_sb = consts.tile([128, KT, 128], F16)
        nc.sync.dma_start(out=wv_sb, in_=wv.rearrange("(kt p) m -> p kt m", p=128))
        wo_sb = consts.tile([128, KT, 128], F16)
        nc.sync.dma_start(out=wo_sb, in_=wo.rearrange("p (jt m) -> p jt m", jt=KT))
        tri_sb = consts.tile([128, 128], F16)
        nc.sync.dma_start(out=tri_sb, in_=tri)
        # ind row 64: ones on cols 0:64 (head A denom -> partitions 0:64)
        # ind row 0: ones on cols 64:128 (head B denom -> partitions 64:128)
        ind_sb = consts.tile([65, 128], F16)
        nc.sync.dma_start(out=ind_sb, in_=ind)
        ident_sb = consts.tile([128, 128], F16)
        nc.sync.dma_start(out=ident_sb, in_=ident)
        biasm1 = consts.tile([128, 1], F32)
        nc.vector.memset(biasm1, -1.0)

        pend_rb = []
        pend_oproj = []

        def pop_rb():
            if pend_rb:
                pend_rb.pop(0)()

        def pop_oproj():
            if pend_oproj:
                pend_oproj.pop(0)()

        batch_tiles = {}

        def alloc_batch(b):
            qT_sb = batch_pool.tile([128, S], F16, name="qT_sb")
            kT_sb = batch_pool.tile([128, S], F16, name="kT_sb")
            vtok_sb = batch_pool.tile([128, NKT, 256], F16, name="vtok_sb")
            nc.gpsimd.memset(vtok_sb[:, :, 64:65], 1.0)
            nc.gpsimd.memset(vtok_sb[:, :, 128:129], 1.0)
            batch_tiles[b] = (qT_sb, kT_sb, vtok_sb)

        def stageA_chunks(b, tt):
            """Projections + v transpose for qtile tt of batch b, as a list
            of small closures to interleave into the attention loop."""
            chunks = []

            def proj(w_sb, done):
                def mm():
                    t0 = b * S + tt * QTILE
                    xt = stA.get("xt")
                    if xt is None:
                        xt = xt_pool.tile([128, KT, QTILE], F16, name="xt", tag="xt")
                        nc.sync.dma_start(
                            out=xt,
                            in_=xT.rearrange("(kt p) t -> p kt t", p=128)[
                                :, :, t0 : t0 + QTILE
                            ],
                        )
                        stA["xt"] = xt
                    p = ps.tile([128, QTILE], F32, name="pP", tag="mm", bufs=2)
                    for kt in range(KT):
                        nc.tensor.matmul(
                            p, w_sb[:, kt, :], xt[:, kt, :],
                            start=(kt == 0), stop=(kt == KT - 1),
                        )
                    done(p)

                chunks.append(mm)

            stA = {}

            def q_done(p):
                qT_sb = batch_tiles[b][0]
                nc.vector.tensor_copy(qT_sb[:, tt * QTILE : (tt + 1) * QTILE], p)

            def k_done(p):
                kT_sb = batch_tiles[b][1]
                nc.vector.tensor_copy(kT_sb[:, tt * QTILE : (tt + 1) * QTILE], p)

            def v_done(p):
                vT_tmp = vtmp_pool.tile([128, QTILE], F16, name="vT_tmp")
                nc.vector.tensor_copy(vT_tmp, p)
                stA["vT"] = vT_tmp

            proj(wq_sb, q_done)
            chunks.append(pop_rb)
            proj(wk_sb, k_done)
            chunks.append(pop_oproj)
            proj(wv_sb, v_done)

            def vtr(sp):
                def run():
                    vtok_sb = batch_tiles[b][2]
                    vT_tmp = stA["vT"]
                    vtokP = ps.tile([128, 2, 128], F16, name="vtokP", tag="mm", bufs=2)
                    for j in range(2):
                        s = sp * 2 + j
                        nc.tensor.transpose(
                            vtokP[:, j, :], vT_tmp[:, s * 128 : (s + 1) * 128], ident_sb
                        )
                    m0 = tt * 4 + sp * 2
                    nc.vector.tensor_copy(
                        vtok_sb[:, m0 : m0 + 2, 0:64], vtokP[:, :, 0:64]
                    )
                    nc.vector.tensor_copy(
                        vtok_sb[:, m0 : m0 + 2, 192:256], vtokP[:, :, 64:128]
                    )

                return run

            for sp in range(QTILE // 256):
                chunks.append(vtr(sp))
            return chunks

        def attention_qtile(b, qi, feed=()):
            """Block-causal attention for q-tile qi of batch b; pops one
            feed chunk per k-iteration to keep the PE saturated. Appends the
            deferred rb/oproj closures."""
            feed = list(feed)
            qT_sb, kT_sb, vtok_sb = batch_tiles[b]
            tb = b * S
            q0 = qi * QTILE  # batch-local q base
            nk = 4 * qi + 4  # k-tiles for this q-tile (block-causal)
            ctxAB = ps.tile([128, 2 * QTILE], F32, name="ctxAB", tag="ctx", bufs=1)

            def geom(m, qi=qi):
                d_off = m - 4 * qi
                if d_off >= 0:
                    return QTILE - 128 * d_off, 128 * d_off, True
                return QTILE, 0, False

            exps = {}
            for i in range(nk + LAG):
                if i >= 1 and i % 2 == 1 and feed:
                    c = feed.pop(0)
                    if c is not None:
                        c()
                if i < nk:
                    m = i
                    width, qoff, diag = geom(m)
                    sAB = ps.tile([128, 2 * QTILE], F32, name="sAB", tag="sc", bufs=2)
                    nc.tensor.matmul(
                        sAB[:, 0:width],
                        kT_sb[0:64, m * 128 : (m + 1) * 128],
                        qT_sb[0:64, q0 + qoff : q0 + QTILE],
                        start=True,
                        stop=True,
                    )
                    nc.tensor.matmul(
                        sAB[:, QTILE : QTILE + width],
                        kT_sb[64:128, m * 128 : (m + 1) * 128],
                        qT_sb[64:128, q0 + qoff : q0 + QTILE],
                        start=True,
                        stop=True,
                    )
                    # one wide exp covers both heads; the gap region
                    # [width:QTILE] holds unread junk for diag tiles.
                    eAB = exp_pool.tile([128, 2 * QTILE], F16, name="eAB", tag="exp")
                    nc.scalar.activation(
                        eAB[:, 0 : QTILE + width],
                        sAB[:, 0 : QTILE + width],
                        EXP_FUNC,
                        scale=INV_SQRT_DK,
                        bias=biasm1,
                    )
                    if diag:
                        nc.vector.tensor_mul(eAB[:, 0:128], eAB[:, 0:128], tri_sb)
                        nc.vector.tensor_mul(
                            eAB[:, QTILE : QTILE + 128],
                            eAB[:, QTILE : QTILE + 128],
                            tri_sb,
                        )
                    exps[m] = eAB

                j = i - LAG
                if j >= 0:
                    width, qoff, _ = geom(j)
                    first = j == 0
                    last = j == nk - 1
                    eAB = exps.pop(j)
                    nc.tensor.matmul(
                        ctxAB[:, qoff:QTILE],
                        vtok_sb[:, j, 0:128],
                        eAB[:, 0:width],
                        start=first,
                        stop=last,
                        skip_group_check=True,
                    )
                    nc.tensor.matmul(
                        ctxAB[:, QTILE + qoff : 2 * QTILE],
                        vtok_sb[:, j, 128:256],
                        eAB[:, QTILE : QTILE + width],
                        start=first,
                        stop=last,
                        skip_group_check=True,
                    )

            for c in feed:
                if c is not None:
                    c()

            # evacuate the two denominator rows: head A at (64, first half),
            # head B at (0, second half). Same-partition copies, fp16-cast.
            s2 = small_pool.tile([65, 2, QTILE], F16, name="s2")
            nc.vector.tensor_copy(s2[64:65, 0, :], ctxAB[64:65, 0:QTILE])
            nc.vector.tensor_copy(s2[0:1, 1, :], ctxAB[0:1, QTILE : 2 * QTILE])
            ctxn = ctxn_pool.tile([128, QTILE], F16, name="ctxn")

            def finish_rb(ctxn=ctxn, s2=s2, ctxAB=ctxAB):
                # broadcast raw denominators via two K=1 fp32r matmuls (ind
                # rows sit on partitions 64 / 0), one fast reciprocal, then
                # normalize fused into the ctx PSUM evacuation copies.
                rbP = ps.tile([128, QTILE], F32, name="rbP", tag="mm", bufs=2)
                nc.tensor.matmul(
                    rbP[0:64, :], ind_sb[64:65, 0:64], s2[64:65, 0, :],
                    start=True, stop=True,
                )
                nc.tensor.matmul(
                    rbP[64:128, :], ind_sb[0:1, 64:128], s2[0:1, 1, :],
                    start=True, stop=True, skip_group_check=True,
                )
                rb_sb = small_pool.tile([128, QTILE], F32, name="rb_sb")
                nc.vector.reciprocal_approx_fast(rb_sb, rbP)
                with nc.allow_low_precision(reason="softmax normalize"):
                    nc.vector.tensor_mul(
                        ctxn[0:64, :], ctxAB[0:64, 0:QTILE], rb_sb[0:64, :]
                    )
                    nc.vector.tensor_mul(
                        ctxn[64:128, :],
                        ctxAB[64:128, QTILE : 2 * QTILE],
                        rb_sb[64:128, :],
                    )

            def finish_oproj(q0=q0, tb=tb, ctxn=ctxn):
                o_all = oall_pool.tile([128, KT, QTILE], F16, name="o_all")
                for jt in range(KT):
                    oP = ps.tile([128, QTILE], F32, name="oP", tag="mm", bufs=2)
                    nc.tensor.matmul(oP, wo_sb[:, jt, :], ctxn, start=True, stop=True)
                    nc.any.tensor_copy(o_all[:, jt, :], oP)
                nc.sync.dma_start(
                    out=outT.rearrange("(jt p) t -> p jt t", p=128)[
                        :, :, tb + q0 : tb + q0 + QTILE
                    ],
                    in_=o_all,
                )

            pend_rb.append(finish_rb)
            pend_oproj.append(finish_oproj)

        # --- schedule: stage A of batch b+1 is interleaved chunk-by-chunk
        # into the attention k-loops of batch b.
        alloc_batch(0)
        for tt in range(NQT):
            for c in stageA_chunks(0, tt):
                if c is not None:
                    c()
        for b in range(B):
            if b + 1 < B:
                alloc_batch(b + 1)
            for qi in range(NQT):
                if b + 1 < B:
                    feed = stageA_chunks(b + 1, qi)
                else:
                    feed = [pop_rb, None, None, pop_oproj]
                attention_qtile(b, qi, feed=feed)

        while pend_rb:
            pend_rb.pop(0)()
        while pend_oproj:
            pend_oproj.pop(0)()

    nc.compile()
    return nc


_NC = None


def _get_nc():
    global _NC
    if _NC is None:
        _NC = build_nc()
    return _NC


def make_in_maps(x, W_Q, W_K, W_V, W_O):
    xTh = np.ascontiguousarray(
        np.asarray(x, dtype=np.float32).reshape(T, D).T
    ).astype(np.float16)
    W_Q = np.asarray(W_Q, dtype=np.float32)
    W_K = np.asarray(W_K, dtype=np.float32)
    W_V = np.asarray(W_V, dtype=np.float32)
    W_O = np.asarray(W_O, dtype=np.float32)
    tri = np.triu(np.ones((128, 128), dtype=np.float16))  # tri[k,q]=1 iff q>=k
    ind2 = np.zeros((65, 128), dtype=np.float16)
    ind2[64, 0:64] = 1.0  # head A denom row (partition 64) -> out parts 0:64
    ind2[0, 64:128] = 1.0  # head B denom row (partition 0) -> out parts 64:128
    ident = np.eye(128, dtype=np.float16)
    in_maps = []
    for c in range(NCORES):
        sl = slice(c * 128, (c + 1) * 128)
        in_maps.append(
            {
                "xT": xTh,
                "wq": np.ascontiguousarray(W_Q[sl, :].T).astype(np.float16),
                "wk": np.ascontiguousarray(W_K[sl, :].T).astype(np.float16),
                "wv": np.ascontiguousarray(W_V[sl, :].T).astype(np.float16),
                "wo": np.ascontiguousarray(W_O.T[sl, :]).astype(np.float16),
                "tri": tri,
                "ind": ind2,
                "ident": ident,
            }
        )
    return in_maps


def kernel(x, W_Q, W_K, W_V, W_O, _results_hook=None):
    nc = _get_nc()
    in_maps = make_in_maps(x, W_Q, W_K, W_V, W_O)
    res = run_bass_kernel_spmd(nc, in_maps, list(range(NCORES)))
    if _results_hook is not None:
        _results_hook(res)
    acc = np.zeros((D, T), dtype=np.float32)
    for c in range(NCORES):
        acc += res.results[c]["outT"].astype(np.float32)
    out = np.ascontiguousarray(acc.T).reshape(B, S, D)
    return out


# revision 10
# speedup vs baseline: 1.0323x; 1.0242x over previous
"""Causal multi-head self-attention on 8 TRN2 NeuronCores.

Sharding: tensor-parallel over heads. 16 heads / 8 cores = 2 heads per core.
Each core computes q/k/v projections for its 2 heads (fp16 matmuls),
block-causal attention (scores kept k-major so softmax sums come from a
fused ones-column in the attn@v matmul), and a partial output projection
against its 128-column slice of W_O. The host sums the 8 fp32 partials.

All matmul operands are fp16 (1.0 cycles/row on the PE, same as wide fp32r,
but half the DMA bytes and LDWEIGHTS time). PSUM accumulation stays fp32.
Scores use K=64 matmuls with partition-base offsets per head (measured on
HW: same per-column rate as K=128, so no zero-padding needed). exp runs on
ScalarE only (scale=1/sqrt(dk), bias=-1 range centering), writing fp16.

vtok layout per k-tile: [vA(0:64) | onesA@64 | .. | onesB@128 | .. |
vB(192:256)], so head A ctx lands on PSUM partitions 0:64 (denom row 64)
and head B ctx on partitions 64:128 (denom row 0) — the ctx evacuation is
two same-partition copies, no cross-partition DMA. The two denominator
rows are fp32r-cast to SBUF, broadcast to [128, QTILE] with two K=1
matmuls (ind rows live on partitions 64 / 0), reciprocal'd with the fast
custom-DVE op, and the normalize multiply runs on the idle Pool engine.

The output projection DMAs PSUM directly to DRAM (fp32) — no engine copy.

Schedule: stage A (projections) of batch b+1 is interleaved per-qtile into
stage B (attention) of batch b, so the PE fills ScalarE-bound exp gaps
with projection matmuls and batch transitions have no PE bubble. Deferred
rb/normalize/oproj of attention tile qi run inside the next interleaved
stage-A qtile.
"""

import numpy as np
from contextlib import ExitStack

import concourse.bass as bass
import concourse.tile as tile
from concourse import bacc, mybir
from concourse.bass_utils import run_bass_kernel_spmd

F32 = mybir.dt.float32
F32R = mybir.dt.float32r
F16 = mybir.dt.float16

B, S, D, H = 4, 2048, 1024, 16
DK = D // H  # 64
NCORES = 8
T = B * S  # 8192 tokens
KT = D // 128  # 8 contraction tiles for projections
QTILE = 512  # q-tile width (tokens)
KTILE = 128  # k-tile width (tokens)
NQT = S // QTILE  # 4 q-tiles per batch
NKT = S // KTILE  # 16 k-tiles per batch
LAG = 2  # ctx matmuls trail scores by this many k-tiles
EXP_FUNC = mybir.ActivationFunctionType.Exp
INV_SQRT_DK = 1.0 / np.sqrt(DK)


def build_nc():
    nc = bacc.Bacc("TRN2", target_bir_lowering=False, debug=False)

    xT = nc.dram_tensor("xT", [D, T], F16, kind="ExternalInput").ap()
    wq = nc.dram_tensor("wq", [D, 128], F16, kind="ExternalInput").ap()
    wk = nc.dram_tensor("wk", [D, 128], F16, kind="ExternalInput").ap()
    wv = nc.dram_tensor("wv", [D, 128], F16, kind="ExternalInput").ap()
    wo = nc.dram_tensor("wo", [128, D], F16, kind="ExternalInput").ap()
    tri = nc.dram_tensor("tri", [128, 128], F16, kind="ExternalInput").ap()
    ind = nc.dram_tensor("ind", [65, 128], F16, kind="ExternalInput").ap()
    ident = nc.dram_tensor("ident", [128, 128], F16, kind="ExternalInput").ap()
    outT = nc.dram_tensor("outT", [D, T], F16, kind="ExternalOutput").ap()

    with ExitStack() as ctx:
        tc = ctx.enter_context(tile.TileContext(nc))
        consts = ctx.enter_context(tc.tile_pool(name="consts", bufs=1))
        xt_pool = ctx.enter_context(tc.tile_pool(name="xt_pool", bufs=2))
        batch_pool = ctx.enter_context(tc.tile_pool(name="batch_pool", bufs=2))
        vtmp_pool = ctx.enter_context(tc.tile_pool(name="vtmp_pool", bufs=3))
        exp_pool = ctx.enter_context(tc.tile_pool(name="exp_pool", bufs=4))
        ctxn_pool = ctx.enter_context(tc.tile_pool(name="ctxn_pool", bufs=2))
        oall_pool = ctx.enter_context(tc.tile_pool(name="oall_pool", bufs=2))
        small_pool = ctx.enter_context(tc.tile_pool(name="small_pool", bufs=2))
        ps = ctx.enter_context(tc.tile_pool(name="ps", bufs=1, space="PSUM"))

        # --- constants / weights (persistent) ---
        wq_sb = consts.tile([128, KT, 128], F16)
        nc.sync.dma_start(out=wq_sb, in_=wq.rearrange("(kt p) m -> p kt m", p=128))
        wk_sb = consts.tile([128, KT, 128], F16)
        nc.sync.dma_start(out=wk_sb, in_=wk.rearrange("(kt p) m -> p kt m", p=128))
        wv_sb = consts.tile([128, KT, 128], F16)
        nc.sync.dma_start(out=wv_sb, in_=wv.rearrange("(kt p) m -> p kt m", p=128))
        wo_sb = consts.tile([128, KT, 128], F16)
        nc.sync.dma_start(out=wo_sb, in_=wo.rearrange("p (jt m) -> p jt m", jt=KT))
        tri_sb = consts.tile([128, 128], F16)
        nc.sync.dma_start(out=tri_sb, in_=tri)
        # ind row 64: ones on cols 0:64 (head A denom -> partitions 0:64)
        # ind row 0: ones on cols 64:128 (head B denom -> partitions 64:128)
        ind_sb = consts.tile([65, 128], F16)
        nc.sync.dma_start(out=ind_sb, in_=ind)
        ident_sb = consts.tile([128, 128], F16)
        nc.sync.dma_start(out=ident_sb, in_=ident)
        biasm1 = consts.tile([128, 1], F32)
        nc.vector.memset(biasm1, -1.0)

        pend_rb = []
        pend_oproj = []

        def pop_rb():
            if pend_rb:
                pend_rb.pop(0)()

        def pop_oproj():
            if pend_oproj:
                pend_oproj.pop(0)()

        batch_tiles = {}

        def alloc_batch(b):
            qT_sb = batch_pool.tile([128, S], F16, name="qT_sb")
            kT_sb = batch_pool.tile([128, S], F16, name="kT_sb")
            vtok_sb = batch_pool.tile([128, NKT, 256], F16, name="vtok_sb")
            nc.gpsimd.memset(vtok_sb[:, :, 64:65], 1.0)
            nc.gpsimd.memset(vtok_sb[:, :, 128:129], 1.0)
            batch_tiles[b] = (qT_sb, kT_sb, vtok_sb)

        def stageA_chunks(b, tt):
            """Projections + v transpose for qtile tt of batch b, as a list
            of small closures to interleave into the attention loop."""
            chunks = []

            def proj(w_sb, done):
                def mm():
                    t0 = b * S + tt * QTILE
                    xt = stA.get("xt")
                    if xt is None:
                        xt = xt_pool.tile([128, KT, QTILE], F16, name="xt", tag="xt")
                        nc.sync.dma_start(
                            out=xt,
                            in_=xT.rearrange("(kt p) t -> p kt t", p=128)[
                                :, :, t0 : t0 + QTILE
                            ],
                        )
                        stA["xt"] = xt
                    p = ps.tile([128, QTILE], F32, name="pP", tag="mm", bufs=2)
                    for kt in range(KT):
                        nc.tensor.matmul(
                            p, w_sb[:, kt, :], xt[:, kt, :],
                            start=(kt == 0), stop=(kt == KT - 1),
                        )
                    done(p)

                chunks.append(mm)

            stA = {}

            def q_done(p):
                qT_sb = batch_tiles[b][0]
                nc.vector.tensor_copy(qT_sb[:, tt * QTILE : (tt + 1) * QTILE], p)

            def k_done(p):
                kT_sb = batch_tiles[b][1]
                nc.vector.tensor_copy(kT_sb[:, tt * QTILE : (tt + 1) * QTILE], p)

            def v_done(p):
                vT_tmp = vtmp_pool.tile([128, QTILE], F16, name="vT_tmp")
                nc.vector.tensor_copy(vT_tmp, p)
                stA["vT"] = vT_tmp

            proj(wq_sb, q_done)
            chunks.append(pop_rb)
            proj(wk_sb, k_done)
            proj(wv_sb, v_done)

            def vtr(sp):
                def run():
                    vtok_sb = batch_tiles[b][2]
                    vT_tmp = stA["vT"]
                    vtokP = ps.tile([128, 2, 128], F16, name="vtokP", tag="mm", bufs=2)
                    for j in range(2):
                        s = sp * 2 + j
                        nc.tensor.transpose(
                            vtokP[:, j, :], vT_tmp[:, s * 128 : (s + 1) * 128], ident_sb
                        )
                    m0 = tt * 4 + sp * 2
                    nc.vector.tensor_copy(
                        vtok_sb[:, m0 : m0 + 2, 0:64], vtokP[:, :, 0:64]
                    )
                    nc.vector.tensor_copy(
                        vtok_sb[:, m0 : m0 + 2, 192:256], vtokP[:, :, 64:128]
                    )

                return run

            for sp in range(QTILE // 256):
                chunks.append(vtr(sp))
            chunks.append(pop_oproj)
            return chunks

        def attention_qtile(b, qi, feed=()):
            """Block-causal attention for q-tile qi of batch b; pops one
            feed chunk per k-iteration to keep the PE saturated. Appends the
            deferred rb/oproj closures."""
            feed = list(feed)
            qT_sb, kT_sb, vtok_sb = batch_tiles[b]
            tb = b * S
            q0 = qi * QTILE  # batch-local q base
            nk = 4 * qi + 4  # k-tiles for this q-tile (block-causal)
            ctxAB = ps.tile([128, 2 * QTILE], F32, name="ctxAB", tag="ctx", bufs=1)

            def geom(m, qi=qi):
                d_off = m - 4 * qi
                if d_off >= 0:
                    return QTILE - 128 * d_off, 128 * d_off, True
                return QTILE, 0, False

            exps = {}
            for i in range(nk + LAG):
                if i >= 1 and i % 2 == 1 and feed:
                    c = feed.pop(0)
                    if c is not None:
                        c()
                if i < nk:
                    m = i
                    width, qoff, diag = geom(m)
                    sAB = ps.tile([128, 2 * QTILE], F32, name="sAB", tag="sc", bufs=2)
                    nc.tensor.matmul(
                        sAB[:, 0:width],
                        kT_sb[0:64, m * 128 : (m + 1) * 128],
                        qT_sb[0:64, q0 + qoff : q0 + QTILE],
                        start=True,
                        stop=True,
                    )
                    nc.tensor.matmul(
                        sAB[:, QTILE : QTILE + width],
                        kT_sb[64:128, m * 128 : (m + 1) * 128],
                        qT_sb[64:128, q0 + qoff : q0 + QTILE],
                        start=True,
                        stop=True,
                    )
                    # one wide exp covers both heads; the gap region
                    # [width:QTILE] holds unread junk for diag tiles.
                    eAB = exp_pool.tile([128, 2 * QTILE], F16, name="eAB", tag="exp")
                    nc.scalar.activation(
                        eAB[:, 0 : QTILE + width],
                        sAB[:, 0 : QTILE + width],
                        EXP_FUNC,
                        scale=INV_SQRT_DK,
                        bias=biasm1,
                    )
                    if diag:
                        nc.vector.tensor_mul(eAB[:, 0:128], eAB[:, 0:128], tri_sb)
                        nc.vector.tensor_mul(
                            eAB[:, QTILE : QTILE + 128],
                            eAB[:, QTILE : QTILE + 128],
                            tri_sb,
                        )
                    exps[m] = eAB

                j = i - LAG
                if j >= 0:
                    width, qoff, _ = geom(j)
                    first = j == 0
                    last = j == nk - 1
                    eAB = exps.pop(j)
                    nc.tensor.matmul(
                        ctxAB[:, qoff:QTILE],
                        vtok_sb[:, j, 0:128],
                        eAB[:, 0:width],
                        start=first,
                        stop=last,
                        skip_group_check=True,
                    )
                    nc.tensor.matmul(
                        ctxAB[:, QTILE + qoff : 2 * QTILE],
                        vtok_sb[:, j, 128:256],
                        eAB[:, QTILE : QTILE + width],
                        start=first,
                        stop=last,
                        skip_group_check=True,
                    )

            for c in feed:
                if c is not None:
                    c()

            # evacuate the two denominator rows: head A at (64, first half),
            # head B at (0, second half). Same-partition copies, fp16-cast.
            s2 = small_pool.tile([65, 2, QTILE], F16, name="s2")
            nc.vector.tensor_copy(s2[64:65, 0, :], ctxAB[64:65, 0:QTILE])
            nc.vector.tensor_copy(s2[0:1, 1, :], ctxAB[0:1, QTILE : 2 * QTILE])
            ctxn = ctxn_pool.tile([128, QTILE], F16, name="ctxn")

            def finish_rb(ctxn=ctxn, s2=s2, ctxAB=ctxAB):
                # broadcast raw denominators via two K=1 fp32r matmuls (ind
                # rows sit on partitions 64 / 0), one fast reciprocal, then
                # normalize fused into the ctx PSUM evacuation copies.
                rbP = ps.tile([128, QTILE], F32, name="rbP", tag="mm", bufs=2)
                nc.tensor.matmul(
                    rbP[0:64, :], ind_sb[64:65, 0:64], s2[64:65, 0, :],
                    start=True, stop=True,
                )
                nc.tensor.matmul(
                    rbP[64:128, :], ind_sb[0:1, 64:128], s2[0:1, 1, :],
                    start=True, stop=True, skip_group_check=True,
                )
                rb_sb = small_pool.tile([128, QTILE], F32, name="rb_sb")
                nc.vector.reciprocal_approx_fast(rb_sb, rbP)
                with nc.allow_low_precision(reason="softmax normalize"):
                    nc.vector.tensor_mul(
                        ctxn[0:64, :], ctxAB[0:64, 0:QTILE], rb_sb[0:64, :]
                    )
                    nc.vector.tensor_mul(
                        ctxn[64:128, :],
                        ctxAB[64:128, QTILE : 2 * QTILE],
                        rb_sb[64:128, :],
                    )

            def finish_oproj(q0=q0, tb=tb, ctxn=ctxn):
                o_all = oall_pool.tile([128, KT, QTILE], F16, name="o_all")
                for jt in range(KT):
                    oP = ps.tile([128, QTILE], F32, name="oP", tag="mm", bufs=2)
                    nc.tensor.matmul(oP, wo_sb[:, jt, :], ctxn, start=True, stop=True)
                    nc.any.tensor_copy(o_all[:, jt, :], oP)
                nc.sync.dma_start(
                    out=outT.rearrange("(jt p) t -> p jt t", p=128)[
                        :, :, tb + q0 : tb + q0 + QTILE
                    ],
                    in_=o_all,
                )

            pend_rb.append(finish_rb)
            pend_oproj.append(finish_oproj)

        # --- schedule: stage A of batch b+1 is interleaved chunk-by-chunk
        # into the attention k-loops of batch b.
        alloc_batch(0)
        for tt in range(NQT):
            for c in stageA_chunks(0, tt):
                if c is not None:
                    c()
        for b in range(B):
            if b + 1 < B:
                alloc_batch(b + 1)
            for qi in range(NQT):
                if b + 1 < B:
                    feed = stageA_chunks(b + 1, qi)
                else:
                    feed = [pop_rb, None, None, pop_oproj]
                attention_qtile(b, qi, feed=feed)

        while pend_rb:
            pend_rb.pop(0)()
        while pend_oproj:
            pend_oproj.pop(0)()

    nc.compile()
    return nc


_NC = None


def _get_nc():
    global _NC
    if _NC is None:
        _NC = build_nc()
    return _NC


def make_in_maps(x, W_Q, W_K, W_V, W_O):
    xTh = np.ascontiguousarray(
        np.asarray(x, dtype=np.float32).reshape(T, D).T
    ).astype(np.float16)
    W_Q = np.asarray(W_Q, dtype=np.float32)
    W_K = np.asarray(W_K, dtype=np.float32)
    W_V = np.asarray(W_V, dtype=np.float32)
    W_O = np.asarray(W_O, dtype=np.float32)
    tri = np.triu(np.ones((128, 128), dtype=np.float16))  # tri[k,q]=1 iff q>=k
    ind2 = np.zeros((65, 128), dtype=np.float16)
    ind2[64, 0:64] = 1.0  # head A denom row (partition 64) -> out parts 0:64
    ind2[0, 64:128] = 1.0  # head B denom row (partition 0) -> out parts 64:128
    ident = np.eye(128, dtype=np.float16)
    in_maps = []
    for c in range(NCORES):
        sl = slice(c * 128, (c + 1) * 128)
        in_maps.append(
            {
                "xT": xTh,
                "wq": np.ascontiguousarray(W_Q[sl, :].T).astype(np.float16),
                "wk": np.ascontiguousarray(W_K[sl, :].T).astype(np.float16),
                "wv": np.ascontiguousarray(W_V[sl, :].T).astype(np.float16),
                "wo": np.ascontiguousarray(W_O.T[sl, :]).astype(np.float16),
                "tri": tri,
                "ind": ind2,
                "ident": ident,
            }
        )
    return in_maps


def kernel(x, W_Q, W_K, W_V, W_O, _results_hook=None):
    nc = _get_nc()
    in_maps = make_in_maps(x, W_Q, W_K, W_V, W_O)
    res = run_bass_kernel_spmd(nc, in_maps, list(range(NCORES)))
    if _results_hook is not None:
        _results_hook(res)
    acc = np.zeros((D, T), dtype=np.float32)
    for c in range(NCORES):
        acc += res.results[c]["outT"].astype(np.float32)
    out = np.ascontiguousarray(acc.T).reshape(B, S, D)
    return out
